# revision 7
# baseline (speedup 1.0000x reference)
"""Multi-head attention layer (B=4, T=S=2048, D=512, H=8) on 8 Trainium2 cores.

Sharding: pure data-parallel over (batch, T-half): core c computes batch c//2,
query rows [512*1024*(c%2) ...]. Weights replicated. Each core runs the full
projections + attention + output projection for its (batch, T-half) shard.

Kernel internals (per core):
  - All big matmuls in float32r (full PE speed, ~2e-4 relative rounding).
  - Activations/weights are PE-transposed on load so every matmul contracts
    over the partition dim.
  - Attention is computed transposed: S.T = K_h @ Q_h.T, so softmax's exp is
    plain elementwise (ACT, straight from PSUM, with the 1/sqrt(HD) scale
    folded in), and P.T feeds the P@V matmul directly with no transposes.
  - Softmax denominators come from a ones-column appended to V per head
    (row 64 of the PV accumulation), normalization happens on the small
    O tensor, not on the 2048-wide P.
  - No max-subtraction in softmax: logits are ~N(0,1) (|logit| < 10), exp is
    safe in fp32 and softmax is shift-invariant so results match reference.
"""

import numpy as np

import concourse.bass as bass
import concourse.tile as tile
from concourse import bacc, mybir
from concourse.bass_utils import run_bass_kernel_spmd
from concourse.masks import make_identity

F32 = mybir.dt.float32
F32R = mybir.dt.float32r
AF = mybir.ActivationFunctionType
OP = mybir.AluOpType

B, T, S, D, H = 4, 2048, 2048, 512, 8
HD = D // H          # 64
NCORES = 8
TSH = T // 2         # 1024 rows of query per core
DC = D // 128        # 4 contraction chunks of 128
ST = S // 128        # 16 key/value tiles of 128
SCALE = 1.0 / np.sqrt(HD)


def build_nc(repeats: int = 1):
    nc = bacc.Bacc("TRN2", target_bir_lowering=False, debug=False)

    q_d = nc.dram_tensor("q", [TSH, D], F32, kind="ExternalInput")
    k_d = nc.dram_tensor("k", [S, D], F32, kind="ExternalInput")
    v_d = nc.dram_tensor("v", [S, D], F32, kind="ExternalInput")
    w_d = {
        n: nc.dram_tensor(n, [D, D], F32, kind="ExternalInput")
        for n in ("wq", "wk", "wv", "wo")
    }
    b_d = {
        n: nc.dram_tensor(n, [D], F32, kind="ExternalInput")
        for n in ("bq", "bk", "bv", "bo")
    }
    out_d = nc.dram_tensor("out", [TSH, D], F32, kind="ExternalOutput")

    with tile.TileContext(nc) as tc:
        with (
            tc.tile_pool(name="const", bufs=1) as const,
            tc.tile_pool(name="wt", bufs=1) as wt,
            tc.tile_pool(name="big", bufs=1) as big,
            tc.tile_pool(name="raw", bufs=2) as rawp,
            tc.tile_pool(name="xt", bufs=2) as xtp,
            tc.tile_pool(name="kblk", bufs=2) as kblkp,
            tc.tile_pool(name="pt", bufs=3) as ptp,
            tc.tile_pool(name="rt", bufs=2) as rtp,
            tc.tile_pool(name="outp", bufs=2) as outp,
            tc.tile_pool(name="tpmm", bufs=2, space="PSUM") as tpmm,
            tc.tile_pool(name="stp", bufs=2, space="PSUM") as stp,
            tc.tile_pool(name="pvp", bufs=2, space="PSUM") as pvp,
        ):
            # ---- constants ----
            ident = const.tile([128, 128], F32)
            make_identity(nc, ident)
            ones_f = const.tile([128, 128], F32)
            nc.vector.memset(ones_f, 1.0)
            ones_r = const.tile([128, 64], F32R)
            nc.vector.tensor_copy(ones_r, ones_f[:, 0:64])

            # per-partition bias views: bq_sb[p, c] = bq[128c + p]
            bq_sb = const.tile([128, DC], F32)
            bk_sb = const.tile([128, DC], F32)
            nc.sync.dma_start(out=bq_sb, in_=b_d["bq"].ap().rearrange("(c p) -> p c", p=128))
            nc.sync.dma_start(out=bk_sb, in_=b_d["bk"].ap().rearrange("(c p) -> p c", p=128))
            # broadcast (over partitions) bias tiles for free-dim biases
            bv_bc = const.tile([128, D], F32)
            bo_bc = const.tile([128, D], F32)
            for bname, btile in (("bv", bv_bc), ("bo", bo_bc)):
                src = b_d[bname].ap()
                bcast = bass.AP(tensor=src.tensor, offset=src.offset,
                                ap=[[0, 128]] + list(src.ap))
                nc.sync.dma_start(out=btile, in_=bcast)

            # ---- weight transposes: WT[p, c, m] = w[m, 128c + p] (f32r) ----
            wts = {}
            for n in ("wq", "wk", "wv", "wo"):
                wts[n] = wt.tile([128, DC, D], F32R, tag=f"wt_{n}", name=f"wt_{n}")
            for n in ("wq", "wk", "wv", "wo"):
                for a in range(4):  # 128-row strips of w
                    raw = rawp.tile([128, D], F32, tag="raw")
                    nc.sync.dma_start(out=raw, in_=w_d[n][128 * a:128 * (a + 1), :])
                    tp = tpmm.tile([128, 4, 128], F32, tag="ps")
                    for c in range(DC):
                        nc.tensor.transpose(tp[:, c, :], raw[:, 128 * c:128 * (c + 1)], ident)
                    nc.vector.tensor_copy(wts[n][:, :, 128 * a:128 * (a + 1)], tp)

            # ---- persistent activations ----
            # Vp[p, st, h, 0:64] = V[st*128+p, 64h+j] + bv ; Vp[..., 64] = 1.0
            vp = big.tile([128, ST, H, HD + 1], F32R, tag="vp")
            kt = big.tile([128, DC, S], F32R, tag="kt")    # K.T (+bk)
            qt = big.tile([128, DC, TSH], F32R, tag="qt")  # Q.T (+bq)
            ot = big.tile([128, DC, TSH], F32R, tag="ot")  # normalized O.T packed

            nc.vector.tensor_copy(
                vp[:, :, :, HD:HD + 1],
                ones_f[:, 0:ST * H].rearrange("p (s h o) -> p s h o", s=ST, h=H))

            for _rep in range(repeats):
                # ---- V projection (natural layout, + ones column) ----
                for st in range(ST):
                    raw = rawp.tile([128, D], F32, tag="raw")
                    nc.sync.dma_start(out=raw, in_=v_d[128 * st:128 * (st + 1), :])
                    tp = tpmm.tile([128, 4, 128], F32, tag="ps")
                    for c in range(DC):
                        nc.tensor.transpose(tp[:, c, :], raw[:, 128 * c:128 * (c + 1)], ident)
                    xt = xtp.tile([128, DC, 128], F32R, tag="xt")
                    nc.vector.tensor_copy(xt, tp)
                    mm = tpmm.tile([128, D], F32, tag="ps")
                    for c in range(DC):
                        nc.tensor.matmul(mm, xt[:, c, :], wts["wv"][:, c, :],
                                         start=(c == 0), stop=(c == DC - 1))
                    nc.vector.tensor_tensor(
                        out=vp[:, st, :, 0:HD],
                        in0=mm.rearrange("p (h d) -> p h d", h=H),
                        in1=bv_bc.rearrange("p (h d) -> p h d", h=H),
                        op=OP.add)

                # ---- K.T / Q.T projections ----
                for name, dram, dst, nblk, bias in (
                    ("k", k_d, kt, S // 512, bk_sb),
                    ("q", q_d, qt, TSH // 512, bq_sb),
                ):
                    for blk in range(nblk):
                        xb = kblkp.tile([128, DC, 512], F32R, tag="kblk")
                        for i in range(4):  # four 128-row tiles per 512 block
                            raw = rawp.tile([128, D], F32, tag="raw")
                            nc.sync.dma_start(
                                out=raw,
                                in_=dram[512 * blk + 128 * i:512 * blk + 128 * (i + 1), :])
                            tp = tpmm.tile([128, 4, 128], F32, tag="ps")
                            for c in range(DC):
                                nc.tensor.transpose(tp[:, c, :], raw[:, 128 * c:128 * (c + 1)], ident)
                            nc.vector.tensor_copy(xb[:, :, 128 * i:128 * (i + 1)], tp)
                        for m in range(DC):  # output dout tiles
                            mm = tpmm.tile([128, 512], F32, tag="ps")
                            for c in range(DC):
                                nc.tensor.matmul(mm, wts["w" + name][:, c, 128 * m:128 * (m + 1)],
                                                 xb[:, c, :],
                                                 start=(c == 0), stop=(c == DC - 1))
                            nc.vector.tensor_scalar_add(
                                dst[:, m, 512 * blk:512 * (blk + 1)], mm, bias[:, m:m + 1])

                # ---- attention per head ----
                for h in range(H):
                    ch, pr = h // 2, 64 * (h % 2)
                    pv0 = pvp.tile([HD + 1, 512], F32, tag="pv")
                    pv1 = pvp.tile([HD + 1, 512], F32, tag="pv")
                    for st in range(ST):
                        sps = stp.tile([128, TSH], F32, tag="st")
                        for tb in range(2):
                            nc.tensor.matmul(
                                sps[:, 512 * tb:512 * (tb + 1)],
                                kt[pr:pr + 64, ch, 128 * st:128 * (st + 1)],
                                qt[pr:pr + 64, ch, 512 * tb:512 * (tb + 1)],
                                start=True, stop=True)
                        pt = ptp.tile([128, TSH], F32R, tag="pt")
                        nc.scalar.activation(pt, sps, AF.Exp, scale=float(SCALE))
                        nc.tensor.matmul(pv0, vp[:, st, h, :], pt[:, 0:512],
                                         start=(st == 0), stop=(st == ST - 1))
                        nc.tensor.matmul(pv1, vp[:, st, h, :], pt[:, 512:1024],
                                         start=(st == 0), stop=(st == ST - 1))
                    for tb, pv in ((0, pv0), (1, pv1)):
                        rt = rtp.tile([128, 512], F32R, tag="rt")
                        with nc.allow_low_precision(reason="recip feeds f32r matmul"):
                            nc.vector.reciprocal(rt[64:65, :], pv[HD:HD + 1, :])
                        bc = tpmm.tile([64, 512], F32, tag="ps")
                        nc.tensor.matmul(bc, ones_r[64:65, :], rt[64:65, :],
                                         start=True, stop=True)
                        bc_sb = rtp.tile([64, 512], F32, tag="bcs")
                        nc.vector.tensor_copy(bc_sb, bc)
                        nc.vector.tensor_tensor(
                            out=ot[pr:pr + 64, ch, 512 * tb:512 * (tb + 1)],
                            in0=pv[0:HD, :], in1=bc_sb, op=OP.mult)

                # ---- output projection ----
                for tt in range(TSH // 128):
                    mm = tpmm.tile([128, D], F32, tag="ps")
                    for c in range(DC):
                        nc.tensor.matmul(mm, ot[:, c, 128 * tt:128 * (tt + 1)],
                                         wts["wo"][:, c, :],
                                         start=(c == 0), stop=(c == DC - 1))
                    ob = outp.tile([128, D], F32, tag="ob")
                    nc.vector.tensor_tensor(out=ob, in0=mm, in1=bo_bc, op=OP.add)
                    nc.sync.dma_start(out=out_d[128 * tt:128 * (tt + 1), :], in_=ob)

    nc.compile()
    return nc


_CACHE = {}


def _get_nc():
    if "nc" not in _CACHE:
        _CACHE["nc"] = build_nc()
    return _CACHE["nc"]


def kernel(query, key, value, Wq, bq, Wk, bk, Wv, bv, Wo, bo):
    f = lambda x: np.ascontiguousarray(np.asarray(x, dtype=np.float32))
    query, key, value = f(query), f(key), f(value)
    shared = {"wq": f(Wq), "wk": f(Wk), "wv": f(Wv), "wo": f(Wo),
              "bq": f(bq), "bk": f(bk), "bv": f(bv), "bo": f(bo)}
    in_maps = []
    for c in range(NCORES):
        b, th = divmod(c, 2)
        in_maps.append({
            "q": query[b, th * TSH:(th + 1) * TSH, :],
            "k": key[b], "v": value[b], **shared,
        })
    nc = _get_nc()
    res = run_bass_kernel_spmd(nc, in_maps, core_ids=list(range(NCORES)))
    out = np.empty((B, T, D), dtype=np.float32)
    for c in range(NCORES):
        b, th = divmod(c, 2)
        out[b, th * TSH:(th + 1) * TSH, :] = res.results[c]["out"]
    return out


# revision 19
# speedup vs baseline: 1.3567x; 1.3567x over previous
"""Multi-head attention layer (B=4, T=S=2048, D=512, H=8) on 8 Trainium2 cores.

Sharding: pure data-parallel over (batch, T-half): core c computes batch c//2,
query rows [1024*(c%2) ...). Weights replicated; no collectives.

The runtime this targets has a large per-instruction dispatch cost, so the
kernel is built around instruction economy and dependency-latency hiding:
  - Transposed operand layouts (x.T, W.T) are produced by strided-AP DMA
    gathers straight from DRAM instead of PE-transpose tile pipelines.
  - All matmuls run in float32r (fp32-width data, full PE rate, ~2e-4
    rounding) with fp32 PSUM accumulation.
  - Attention is computed transposed (S.T = K_h @ Q_h.T) so softmax exp is a
    plain elementwise ACT op straight from PSUM (scale=1/sqrt(HD) folded in),
    and P.T feeds P@V directly with no transposes.
  - Softmax denominators come from a ones-column appended to V per head (row
    64 of the PV accumulation); normalization happens on the small O tensor.
    No max-subtraction: logits are ~N(0,1), exp is safe in fp32 envelope and
    softmax is shift-invariant, so results match the reference.
  - Within each head, P@V matmuls trail the S.T matmuls by a few chunk slots
    so the PE never waits on the ACT exp round trip.
"""

from contextlib import ExitStack

import numpy as np

import concourse.bass as bass
import concourse.tile as tile
from concourse import bacc, mybir
from concourse.bass_utils import run_bass_kernel_spmd

F32 = mybir.dt.float32
F32R = mybir.dt.float32r
AF = mybir.ActivationFunctionType
OP = mybir.AluOpType

B, T, S, D, H = 4, 2048, 2048, 512, 8
HD = D // H          # 64
NCORES = 8
TSH = T // 2         # 1024 query rows per core
DC = D // 128        # 4 contraction chunks
ST = S // 128        # 16 key/value s-tiles
SCALE = 1.0 / np.sqrt(HD)
LOOK = 6             # P@V trails S.T by this many s-chunk slots


def build_nc(repeats: int = 1):
    MOV = 512  # matmul N limit: one fp32 PSUM bank per matmul

    nc = bacc.Bacc("TRN2", target_bir_lowering=False, debug=False)

    q_d = nc.dram_tensor("q", [TSH, D], F32, kind="ExternalInput")
    k_d = nc.dram_tensor("k", [S, D], F32, kind="ExternalInput")
    v_d = nc.dram_tensor("v", [S, D], F32, kind="ExternalInput")
    w_d = {n: nc.dram_tensor(n, [D, D], F32, kind="ExternalInput")
           for n in ("wq", "wk", "wv", "wo")}
    b_d = {n: nc.dram_tensor(n, [D], F32, kind="ExternalInput")
           for n in ("bq", "bk", "bv", "bo")}
    out_d = nc.dram_tensor("out", [TSH, D], F32, kind="ExternalOutput")

    with tile.TileContext(nc) as tc, ExitStack() as top:
        const = top.enter_context(tc.tile_pool(name="const", bufs=1))
        wt = top.enter_context(tc.tile_pool(name="wt", bufs=1))
        big = top.enter_context(tc.tile_pool(name="big", bufs=1))
        stp = top.enter_context(tc.tile_pool(name="stps", bufs=3, space="PSUM"))
        pvp = top.enter_context(tc.tile_pool(name="pvps", bufs=1, space="PSUM"))

        # ---- constants ----
        ones_f = const.tile([128, 64], F32)
        nc.vector.memset(ones_f, 1.0)
        ones_r = const.tile([128, 64], F32R)
        nc.vector.tensor_copy(ones_r, ones_f)

        bq_sb = const.tile([128, DC], F32)
        bk_sb = const.tile([128, DC], F32)
        nc.sync.dma_start(out=bq_sb, in_=b_d["bq"].ap().rearrange("(c p) -> p c", p=128))
        nc.sync.dma_start(out=bk_sb, in_=b_d["bk"].ap().rearrange("(c p) -> p c", p=128))
        bv_bc = const.tile([128, D], F32)
        bo_bc = const.tile([128, D], F32)
        for bname, btile in (("bv", bv_bc), ("bo", bo_bc)):
            src = b_d[bname].ap()
            nc.sync.dma_start(out=btile, in_=bass.AP(
                tensor=src.tensor, offset=src.offset, ap=[[0, 128]] + list(src.ap)))

        # ---- persistent tensors (f32r) ----
        wts = {n: wt.tile([128, DC, D], F32R, tag=f"wt_{n}", name=f"wt_{n}")
               for n in ("wq", "wk", "wv", "wo")}
        vp = big.tile([128, ST, H, HD + 1], F32R, tag="vp", name="vp")
        kt = big.tile([128, DC, S], F32R, tag="kt", name="kt")
        qt = big.tile([128, DC, TSH], F32R, tag="qt", name="qt")

        ones_a = const.tile([128, ST * H], F32)
        nc.vector.memset(ones_a, 1.0)
        nc.vector.tensor_copy(
            vp[:, :, :, HD:HD + 1],
            ones_a.rearrange("p (s h o) -> p s h o", s=ST, h=H))

        for _rep in range(repeats):
            # ======== load + project phase (scoped pools) ========
            with ExitStack() as ph:
                loadp = ph.enter_context(tc.tile_pool(name="load", bufs=2))
                stagep = ph.enter_context(tc.tile_pool(name="stage", bufs=1))

                # transposed weights via DMA gather + cast: wT[p,c,o] = w[o,128c+p]
                for n in ("wv", "wk", "wq", "wo"):
                    stg = stagep.tile([128, DC, D], F32, tag="stgw", name=f"stg_{n}")
                    wview = w_d[n].ap().rearrange("o (c p) -> p c o", p=128)
                    for c in range(DC):
                        nc.sync.dma_start(out=stg[:, c, :], in_=wview[:, c, :])
                    nc.vector.tensor_copy(wts[n], stg)

                def load_half(dram, n_rows, half, name):
                    """Gather x.T[p, c, t] for t in one half, cast to f32r."""
                    lo, hi = half * n_rows // 2, (half + 1) * n_rows // 2
                    tview = dram.ap().rearrange("t (c p) -> p c t", p=128)
                    stg = stagep.tile([128, DC, n_rows // 2], F32, tag="stgx",
                                      name=f"stg_{name}")
                    for c in range(DC):
                        nc.sync.dma_start(out=stg[:, c, :], in_=tview[:, c, lo:hi])
                    xr = loadp.tile([128, DC, n_rows // 2], F32R, tag="xr",
                                    name=f"xr_{name}")
                    nc.vector.tensor_copy(xr, stg)
                    return xr

                # V' projection: vp[p, st, h, 0:64] = (v @ Wv.T + bv), col 64 = 1
                for half in range(2):
                    vT = load_half(v_d, S, half, f"v{half}")
                    for sti in range(ST // 2):
                        st = half * (ST // 2) + sti
                        mm = stp.tile([128, D], F32, tag="st", name=f"vmm{st}")
                        for c in range(DC):
                            nc.tensor.matmul(
                                mm, vT[:, c, 128 * sti:128 * (sti + 1)],
                                wts["wv"][:, c, :],
                                start=(c == 0), stop=(c == DC - 1))
                        nc.vector.tensor_tensor(
                            out=vp[:, st, :, 0:HD],
                            in0=mm.rearrange("p (h d) -> p h d", h=H),
                            in1=bv_bc.rearrange("p (h d) -> p h d", h=H), op=OP.add)

                # K.T / Q.T projections
                for wname, dram, dst, n_cols, bias in (
                    ("wk", k_d, kt, S, bk_sb), ("wq", q_d, qt, TSH, bq_sb),
                ):
                    for half in range(2):
                        xT = load_half(dram, n_cols, half, f"{wname}{half}")
                        for m in range(DC):
                            for blk in range(n_cols // 2 // MOV):
                                off = half * n_cols // 2 + MOV * blk
                                mm = stp.tile([128, MOV], F32, tag="st", name="pmm")
                                for c in range(DC):
                                    nc.tensor.matmul(
                                        mm, wts[wname][:, c, 128 * m:128 * (m + 1)],
                                        xT[:, c, MOV * blk:MOV * (blk + 1)],
                                        start=(c == 0), stop=(c == DC - 1))
                                nc.vector.tensor_scalar_add(
                                    dst[:, m, off:off + MOV], mm, bias[:, m:m + 1])

            # ======== attention + output projection (scoped pools) ========
            with ExitStack() as ph:
                attnp = ph.enter_context(tc.tile_pool(name="attnp", bufs=1))
                ptp = ph.enter_context(tc.tile_pool(name="pt", bufs=LOOK + 2))
                rtp = ph.enter_context(tc.tile_pool(name="rt", bufs=1))
                ovp = ph.enter_context(tc.tile_pool(name="ov", bufs=1))
                outp = ph.enter_context(tc.tile_pool(name="outp", bufs=2))

                ot = attnp.tile([128, DC, TSH], F32R, tag="ot", name="ot")
                n_tb = TSH // MOV

                for h in range(H):
                    ch, pr = h // 2, 64 * (h % 2)
                    pts = {}
                    pv = pvp.tile([HD + 1, TSH], F32, tag="pv", name=f"pv{h}")

                    def pv_chunk(st):
                        pt = pts.pop(st)
                        for tb in range(n_tb):
                            nc.tensor.matmul(
                                pv[:, MOV * tb:MOV * (tb + 1)],
                                vp[:, st, h, :], pt[:, MOV * tb:MOV * (tb + 1)],
                                start=(st == 0), stop=(st == ST - 1))

                    for st in range(ST):
                        sps = stp.tile([128, TSH], F32, tag="st", name=f"sps{h}_{st}")
                        for tb in range(n_tb):
                            nc.tensor.matmul(
                                sps[:, MOV * tb:MOV * (tb + 1)],
                                kt[pr:pr + 64, ch, 128 * st:128 * (st + 1)],
                                qt[pr:pr + 64, ch, MOV * tb:MOV * (tb + 1)],
                                start=True, stop=True)
                        pt = ptp.tile([128, TSH], F32R, tag="pt", name=f"pt{h}_{st}")
                        nc.scalar.activation(pt, sps, AF.Exp, scale=float(SCALE))
                        pts[st] = pt
                        if st >= LOOK:
                            pv_chunk(st - LOOK)
                    for st in range(ST - LOOK, ST):
                        pv_chunk(st)

                    # normalize: ot[head rows] = pv[0:64] * (1 / denom row)
                    ov = ovp.tile([HD, TSH], F32, tag="ov", name=f"ov{h}")
                    nc.vector.tensor_copy(ov, pv[0:HD, :])
                    rt = rtp.tile([65, TSH], F32R, tag="rt", name=f"rt{h}")
                    with nc.allow_low_precision(reason="recip feeds f32r matmul"):
                        nc.vector.reciprocal(rt[64:65, :], pv[HD:HD + 1, :])
                    bc = stp.tile([64, TSH], F32, tag="st", name=f"bc{h}")
                    for rb in range(TSH // 512):
                        nc.tensor.matmul(bc[:, 512 * rb:512 * (rb + 1)],
                                         ones_r[64:65, :],
                                         rt[64:65, 512 * rb:512 * (rb + 1)],
                                         start=True, stop=True)
                    nc.vector.tensor_tensor(out=ot[pr:pr + 64, ch, :], in0=ov,
                                            in1=bc, op=OP.mult)

                # ---- output projection ----
                for tt in range(TSH // 128):
                    mm = stp.tile([128, D], F32, tag="st", name=f"omm{tt}")
                    for c in range(DC):
                        nc.tensor.matmul(mm, ot[:, c, 128 * tt:128 * (tt + 1)],
                                         wts["wo"][:, c, :],
                                         start=(c == 0), stop=(c == DC - 1))
                    ob = outp.tile([128, D], F32, tag="ob", name=f"ob{tt}")
                    nc.vector.tensor_tensor(out=ob, in0=mm, in1=bo_bc, op=OP.add)
                    nc.sync.dma_start(out=out_d[128 * tt:128 * (tt + 1), :], in_=ob)

    nc.compile()
    return nc


_CACHE = {}


def _get_nc():
    if "nc" not in _CACHE:
        _CACHE["nc"] = build_nc()
    return _CACHE["nc"]


def kernel(query, key, value, Wq, bq, Wk, bk, Wv, bv, Wo, bo):
    f = lambda x: np.ascontiguousarray(np.asarray(x, dtype=np.float32))
    query, key, value = f(query), f(key), f(value)
    shared = {"wq": f(Wq), "wk": f(Wk), "wv": f(Wv), "wo": f(Wo),
              "bq": f(bq), "bk": f(bk), "bv": f(bv), "bo": f(bo)}
    in_maps = []
    for c in range(NCORES):
        b, th = divmod(c, 2)
        in_maps.append({
            "q": query[b, th * TSH:(th + 1) * TSH, :],
            "k": key[b], "v": value[b], **shared,
        })
    nc = _get_nc()
    res = run_bass_kernel_spmd(nc, in_maps, core_ids=list(range(NCORES)))
    out = np.empty((B, T, D), dtype=np.float32)
    for c in range(NCORES):
        b, th = divmod(c, 2)
        out[b, th * TSH:(th + 1) * TSH, :] = res.results[c]["out"]
    return out


# revision 26
# speedup vs baseline: 4.7801x; 3.5232x over previous
"""Multi-head attention layer (B=4, T=S=2048, D=512, H=8) on 8 Trainium2 cores.

Sharding: pure data-parallel over (batch, T-half): core c computes batch c//2,
query rows [1024*(c%2) ...). Weights replicated; no collectives.

The runtime this targets has a large per-instruction dispatch cost, so the
kernel is built around instruction economy and dependency-latency hiding:
  - Transposed operand layouts (x.T, W.T) are produced by strided-AP DMA
    gathers straight from DRAM instead of PE-transpose tile pipelines.
  - All matmuls run in float32r (fp32-width data, full PE rate, ~2e-4
    rounding) with fp32 PSUM accumulation.
  - Attention is computed transposed (S.T = K_h @ Q_h.T) so softmax exp is a
    plain elementwise ACT op straight from PSUM (scale=1/sqrt(HD) folded in),
    and P.T feeds P@V directly with no transposes.
  - Softmax denominators come from a ones-column appended to V per head (row
    64 of the PV accumulation); normalization happens on the small O tensor.
    No max-subtraction: logits are ~N(0,1), exp is safe in fp32 envelope and
    softmax is shift-invariant, so results match the reference.
  - Within each head, P@V matmuls trail the S.T matmuls by a few chunk slots
    so the PE never waits on the ACT exp round trip.
"""

from contextlib import ExitStack

import numpy as np

import concourse.bass as bass
import concourse.tile as tile
from concourse import bacc, mybir
from concourse.bass_utils import run_bass_kernel_spmd

F32 = mybir.dt.float32
F32R = mybir.dt.float32r
AF = mybir.ActivationFunctionType
OP = mybir.AluOpType

B, T, S, D, H = 4, 2048, 2048, 512, 8
HD = D // H          # 64
NCORES = 8
TSH = T // 2         # 1024 query rows per core
DC = D // 128        # 4 contraction chunks
ST = S // 128        # 16 key/value s-tiles
SCALE = 1.0 / np.sqrt(HD)
LOOK = 6             # P@V trails S.T by this many s-chunk slots


def build_nc(repeats: int = 1, stages: str = "lpao"):
    MOV = 512  # matmul N limit: one fp32 PSUM bank per matmul

    nc = bacc.Bacc("TRN2", target_bir_lowering=False, debug=False)

    q_d = nc.dram_tensor("q", [TSH, D], F32, kind="ExternalInput")
    k_d = nc.dram_tensor("k", [S, D], F32, kind="ExternalInput")
    v_d = nc.dram_tensor("v", [S, D], F32, kind="ExternalInput")
    w_d = {n: nc.dram_tensor(n, [D, D], F32, kind="ExternalInput")
           for n in ("wq", "wk", "wv", "wo")}
    b_d = {n: nc.dram_tensor(n, [D], F32, kind="ExternalInput")
           for n in ("bq", "bk", "bv", "bo")}
    out_d = nc.dram_tensor("out", [TSH, D], F32, kind="ExternalOutput")

    with tile.TileContext(nc) as tc, ExitStack() as top:
        const = top.enter_context(tc.tile_pool(name="const", bufs=1))
        wt = top.enter_context(tc.tile_pool(name="wt", bufs=1))
        big = top.enter_context(tc.tile_pool(name="big", bufs=1))
        stp = top.enter_context(tc.tile_pool(name="stps", bufs=2, space="PSUM"))
        pvp = top.enter_context(tc.tile_pool(name="pvps", bufs=4, space="PSUM"))

        # ---- constants ----
        ones_f = const.tile([128, 64], F32)
        nc.vector.memset(ones_f, 1.0)
        ones_r = const.tile([128, 64], F32R)
        nc.vector.tensor_copy(ones_r, ones_f)

        bq_sb = const.tile([128, DC], F32)
        bk_sb = const.tile([128, DC], F32)
        nc.sync.dma_start(out=bq_sb, in_=b_d["bq"].ap().rearrange("(c p) -> p c", p=128))
        nc.sync.dma_start(out=bk_sb, in_=b_d["bk"].ap().rearrange("(c p) -> p c", p=128))
        bv_bc = const.tile([128, D], F32)
        bo_bc = const.tile([128, D], F32)
        for bname, btile in (("bv", bv_bc), ("bo", bo_bc)):
            src = b_d[bname].ap()
            nc.sync.dma_start(out=btile, in_=bass.AP(
                tensor=src.tensor, offset=src.offset, ap=[[0, 128]] + list(src.ap)))

        # ---- persistent tensors (f32r) ----
        wts = {"wo": wt.tile([128, DC, D], F32R, tag="wt_wo", name="wt_wo")}
        vp = big.tile([128, ST, H, HD + 1], F32R, tag="vp", name="vp")
        kt = big.tile([128, DC, S], F32R, tag="kt", name="kt")
        qt = big.tile([128, DC, TSH], F32R, tag="qt", name="qt")

        ones_a = const.tile([128, ST * H], F32)
        nc.vector.memset(ones_a, 1.0)
        nc.vector.tensor_copy(
            vp[:, :, :, HD:HD + 1],
            ones_a.rearrange("p (s h o) -> p s h o", s=ST, h=H))

        for _rep in range(repeats):
            # ======== load + project phase (scoped pools) ========
            with ExitStack() as ph:
                loadp = ph.enter_context(tc.tile_pool(name="load", bufs=2))
                stagep = ph.enter_context(tc.tile_pool(name="stage", bufs=1))
                wtp = ph.enter_context(tc.tile_pool(name="wtp", bufs=1))
                for n in ("wq", "wk", "wv"):
                    wts[n] = wtp.tile([128, DC, D], F32R, tag=f"wt_{n}",
                                      name=f"wt_{n}")

                # transposed weights via DMA gather + cast: wT[p,c,o] = w[o,128c+p]
                for n in (("wv", "wk", "wq", "wo") if "l" in stages else ()):
                    stg = stagep.tile([128, DC, D], F32, tag="stgw", name=f"stg_{n}")
                    wview = w_d[n].ap().rearrange("o (c p) -> p c o", p=128)
                    for c in range(DC):
                        nc.sync.dma_start(out=stg[:, c, :], in_=wview[:, c, :])
                    nc.vector.tensor_copy(wts[n], stg)

                def load_half(dram, n_rows, half, name):
                    """Gather x.T[p, c, t] for t in one half, cast to f32r."""
                    lo, hi = half * n_rows // 2, (half + 1) * n_rows // 2
                    tview = dram.ap().rearrange("t (c p) -> p c t", p=128)
                    stg = stagep.tile([128, DC, n_rows // 2], F32, tag="stgx",
                                      name=f"stg_{name}")
                    for c in range(DC):
                        nc.sync.dma_start(out=stg[:, c, :], in_=tview[:, c, lo:hi])
                    xr = loadp.tile([128, DC, n_rows // 2], F32R, tag="xr",
                                    name=f"xr_{name}")
                    nc.vector.tensor_copy(xr, stg)
                    return xr

                # V' projection
                for half in range(2 if "p" in stages else 0):
                    vT = load_half(v_d, S, half, f"v{half}")
                    for sti in range(ST // 2):
                        st = half * (ST // 2) + sti
                        mm = stp.tile([128, D], F32, tag="st", name=f"vmm{st}")
                        for c in range(DC):
                            nc.tensor.matmul(
                                mm, vT[:, c, 128 * sti:128 * (sti + 1)],
                                wts["wv"][:, c, :],
                                start=(c == 0), stop=(c == DC - 1))
                        nc.vector.tensor_tensor(
                            out=vp[:, st, :, 0:HD],
                            in0=mm.rearrange("p (h d) -> p h d", h=H),
                            in1=bv_bc.rearrange("p (h d) -> p h d", h=H), op=OP.add)

                # K.T / Q.T projections
                for wname, dram, dst, n_cols, bias in (
                    ("wk", k_d, kt, S, bk_sb), ("wq", q_d, qt, TSH, bq_sb),
                ):
                    for half in range(2 if "p" in stages else 0):
                        xT = load_half(dram, n_cols, half, f"{wname}{half}")
                        for m in range(DC):
                            for blk in range(n_cols // 2 // MOV):
                                off = half * n_cols // 2 + MOV * blk
                                mm = stp.tile([128, MOV], F32, tag="st", name="pmm")
                                for c in range(DC):
                                    nc.tensor.matmul(
                                        mm, wts[wname][:, c, 128 * m:128 * (m + 1)],
                                        xT[:, c, MOV * blk:MOV * (blk + 1)],
                                        start=(c == 0), stop=(c == DC - 1))
                                nc.vector.tensor_scalar_add(
                                    dst[:, m, off:off + MOV], mm, bias[:, m:m + 1])

            # ======== attention + output projection (scoped pools) ========
            with ExitStack() as ph:
                attnp = ph.enter_context(tc.tile_pool(name="attnp", bufs=1))
                ptp = ph.enter_context(tc.tile_pool(name="pt", bufs=14))
                rtp = ph.enter_context(tc.tile_pool(name="rt", bufs=2))
                ovp = ph.enter_context(tc.tile_pool(name="ov", bufs=4))
                outp = ph.enter_context(tc.tile_pool(name="outp", bufs=2))

                ot = attnp.tile([128, DC, TSH], F32R, tag="ot", name="ot")
                n_tb = TSH // MOV

                for h in range(H if ("a" in stages or "s" in stages) else 0):
                    ch, pr = h // 2, 64 * (h % 2)
                    pts = {}
                    # 4 independent accumulation chains: (t-half, chunk parity)
                    pvs = {(tb, par): pvp.tile([HD + 1, MOV], F32, tag="pv",
                                               name=f"pv{h}_{tb}_{par}")
                           for tb in range(n_tb) for par in range(2)}

                    def pv_chunk(st):
                        pt = pts.pop(st)
                        for tb in range(n_tb):
                            nc.tensor.matmul(
                                pvs[(tb, st % 2)],
                                vp[:, st, h, :], pt[:, MOV * tb:MOV * (tb + 1)],
                                start=(st < 2), stop=(st >= ST - 2))

                    for st in range(ST):
                        sps = stp.tile([128, TSH], F32, tag="st", name=f"sps{h}_{st}")
                        for tb in range(n_tb):
                            nc.tensor.matmul(
                                sps[:, MOV * tb:MOV * (tb + 1)],
                                kt[pr:pr + 64, ch, 128 * st:128 * (st + 1)],
                                qt[pr:pr + 64, ch, MOV * tb:MOV * (tb + 1)],
                                start=True, stop=True)
                        pt = ptp.tile([128, TSH], F32R, tag="pt", name=f"pt{h}_{st}")
                        nc.scalar.activation(pt, sps, AF.Exp, scale=float(SCALE))
                        pts[st] = pt
                        if "a" in stages and st >= LOOK:
                            pv_chunk(st - LOOK)
                    if "a" not in stages:
                        pts.clear()
                        continue
                    for st in range(ST - LOOK, ST):
                        pv_chunk(st)

                    # combine chain pairs + normalize by the denominator row
                    for tb in range(n_tb):
                        cmb = ovp.tile([HD + 1, MOV], F32, tag="ov", name=f"cb{h}_{tb}")
                        nc.vector.tensor_copy(cmb, pvs[(tb, 0)])
                        cm2 = ovp.tile([HD + 1, MOV], F32, tag="ov", name=f"cm{h}_{tb}")
                        nc.vector.tensor_tensor(out=cm2, in0=cmb, in1=pvs[(tb, 1)],
                                                op=OP.add)
                        rt = rtp.tile([65, MOV], F32R, tag="rt", name=f"rt{h}_{tb}")
                        with nc.allow_low_precision(reason="recip feeds f32r matmul"):
                            nc.vector.reciprocal(rt[64:65, :], cm2[HD:HD + 1, :])
                        bc = stp.tile([64, MOV], F32, tag="st", name=f"bc{h}_{tb}")
                        nc.tensor.matmul(bc, ones_r[64:65, :], rt[64:65, :],
                                         start=True, stop=True)
                        nc.vector.tensor_tensor(
                            out=ot[pr:pr + 64, ch, MOV * tb:MOV * (tb + 1)],
                            in0=cm2[0:HD, :], in1=bc, op=OP.mult)

                # ---- output projection ----
                for tt in range(TSH // 128 if "o" in stages else 0):
                    mm = stp.tile([128, D], F32, tag="st", name=f"omm{tt}")
                    for c in range(DC):
                        nc.tensor.matmul(mm, ot[:, c, 128 * tt:128 * (tt + 1)],
                                         wts["wo"][:, c, :],
                                         start=(c == 0), stop=(c == DC - 1))
                    ob = outp.tile([128, D], F32, tag="ob", name=f"ob{tt}")
                    nc.vector.tensor_tensor(out=ob, in0=mm, in1=bo_bc, op=OP.add)
                    nc.sync.dma_start(out=out_d[128 * tt:128 * (tt + 1), :], in_=ob)

    nc.compile()
    return nc


_CACHE = {}


def _get_nc():
    if "nc" not in _CACHE:
        _CACHE["nc"] = build_nc()
    return _CACHE["nc"]


def kernel(query, key, value, Wq, bq, Wk, bk, Wv, bv, Wo, bo):
    f = lambda x: np.ascontiguousarray(np.asarray(x, dtype=np.float32))
    query, key, value = f(query), f(key), f(value)
    shared = {"wq": f(Wq), "wk": f(Wk), "wv": f(Wv), "wo": f(Wo),
              "bq": f(bq), "bk": f(bk), "bv": f(bv), "bo": f(bo)}
    in_maps = []
    for c in range(NCORES):
        b, th = divmod(c, 2)
        in_maps.append({
            "q": query[b, th * TSH:(th + 1) * TSH, :],
            "k": key[b], "v": value[b], **shared,
        })
    nc = _get_nc()
    res = run_bass_kernel_spmd(nc, in_maps, core_ids=list(range(NCORES)))
    out = np.empty((B, T, D), dtype=np.float32)
    for c in range(NCORES):
        b, th = divmod(c, 2)
        out[b, th * TSH:(th + 1) * TSH, :] = res.results[c]["out"]
    return out
